# revision 39
# baseline (speedup 1.0000x reference)
"""Trainium2 Bass kernel for nn_MultiHeadAttentionLayer (edge-wise MHA with
global softmax over the edge dimension).

Strategy (8 NeuronCores, data-parallel over edges):
  - Host shards E=250000 edges into 8 shards of 31250, zero-padded to 31744
    (62 chunks x 512), pre-transposed so features land on SBUF partitions,
    and cast to bf16.  eaT carries 33 rows: 32 edge features + a ones-row
    (1.0 on valid cols, 0.0 on pad) whose weight row is bk+be, folding the
    K/edge bias into the KE matmul for free.
  - Phase A1 (scores only, 4 matmuls/chunk): QT = wq.T@xiT, KET =
    wk.T@xjT + wea33.T@eaT (bias included).  KE copies to SBUF (ACT 2/5
    of chunks, DVE 3/5 -- DVE can read only ONE PSUM operand per op, and
    GPSIMD none), then one DVE scalar_tensor_tensor P = (QT+bq)*KET.
    S = HsumRep.T@P is deferred THREE chunks so the in-order PE never waits
    on the 2-hop copy->multiply chain (pq/pke PSUM rings are 3 deep).
    exp(S/4) per chunk -> resident SBUF bf16 e_full + per-chunk Z partials
    via the ACT accumulator.  x_j streams into a RESIDENT SBUF buffer
    (reused in A2).  Pad cols give exp(0)=1 exactly, so Z is fixed by
    subtracting the compile-time constant 8*494 after the AllReduce --
    no tail special-casing, the AR triggers ~3us after the last matmul.
  - AllReduce(add) of Z[128,1] from the GPSIMD queue (the collective
    BLOCKS its issuing queue until completion, so A2 avoids GPSIMD).
    While the AR is in flight (11.5us fixed trigger delay + 9-40us, high
    variance on this stack), phase A2 runs: VT = wv.T@xjT per chunk,
    U = (VT+bv)*e_full written IN PLACE over the resident x_j buffer.
    3/8 of chunks: DVE scalar_tensor_tensor straight from PSUM; 5/8: ACT
    copies V+bv to SBUF and DVE does the all-bf16 multiply at 2x rate.
  - 1/(Z-pad) is folded into wo's rows (wo2 = wo * chd).  Pass B (per
    chunk pair): outT = wo2.T@U + bo -> DRAM fp16; the PSUM->SBUF copy
    alternates ACT/DVE; po ring is 4 pairs deep (all 8 PSUM banks); the
    phase is output-DMA-bound (~260 GB/s per core on this stack).
  - Host gathers and transposes back to [E, 128].
  Measured: 206us (prev session baseline) -> 167-178us (AllReduce
  duration is a 9-43us lottery on this stack); rel err 2.3e-4.
  Tried and rejected: warm-up AllReduce (real AR duration balloons 2-4x),
  collective on the sync queue (BIR: Pool/DMA engines only), GPSIMD for
  any PSUM read (illegal), GPSIMD free-axis reduce (partition-axis only),
  x_j loads via GPSIMD software DGE (slower, starved A1).
"""
import os
import sys

for _p in ("/opt/trn_rl_repo", "/root/.axon_site/_ro/trn_rl_repo"):
    if os.path.isdir(_p) and _p not in sys.path:
        sys.path.append(_p)

import numpy as np
import ml_dtypes
import concourse.bacc as bacc
import concourse.tile as tile
import concourse.mybir as mybir
from concourse.bass_utils import run_bass_kernel_spmd

F32 = mybir.dt.float32
BF16 = mybir.dt.bfloat16
AF = mybir.ActivationFunctionType
ALU = mybir.AluOpType
BF = ml_dtypes.bfloat16

E_FULL = 250000
NCORES = 8
ES = E_FULL // NCORES          # 31250 edges per core
CH = 512                       # chunk size (PSUM bank width)
NCH = (ES + CH - 1) // CH      # 62 chunks
EP = NCH * CH                  # 31744 padded edges per core
D = 128
NH = 8
DK = 16
XW = 4096                      # DMA batch width (8 chunks)
NPAIR = NCH // 2               # 31 exp pairs
PADZ = float(NCORES * (EP - ES))   # exp(0)=1 per pad col: global Z excess

_CACHE = {}


def _build():
    if "nc" in _CACHE:
        return _CACHE["nc"]

    nc = bacc.Bacc(num_devices=NCORES)

    t_xiT = nc.dram_tensor("xiT", [D, EP], BF16, kind="ExternalInput")
    t_xjT = nc.dram_tensor("xjT", [D, EP], BF16, kind="ExternalInput")
    t_eaT = nc.dram_tensor("eaT", [33, EP], BF16, kind="ExternalInput")
    t_pkb = nc.dram_tensor("pkb", [D, 768], BF16, kind="ExternalInput")
    t_pkf = nc.dram_tensor("pkf", [D, 8], F32, kind="ExternalInput")
    t_out = nc.dram_tensor("outT", [D, EP], mybir.dt.float16, kind="ExternalOutput")

    with tile.TileContext(nc) as tc:
        with (
            tc.tile_pool(name="per", bufs=1) as per,      # persistent
            tc.tile_pool(name="wk", bufs=2) as wk,        # streaming loads
            tc.tile_pool(name="mid", bufs=2) as mid,      # intermediates
            tc.tile_pool(name="dram", bufs=1, space="DRAM") as dram,
        ):
            s_pkb = per.tile([D, 768], BF16)
            nc.scalar.dma_start(s_pkb[:], t_pkb[:])
            s_wq = s_pkb[:, 0:128]
            s_wk = s_pkb[:, 128:256]
            s_wv = s_pkb[:, 256:384]
            s_wo = s_pkb[:, 384:512]
            s_wea = s_pkb[0:33, 512:640]     # [we; bk+be]
            s_hrep = s_pkb[:, 640:768]       # HsumRep [f, hd]

            s_pkf = per.tile([D, 8], F32)
            nc.scalar.dma_start(s_pkf[:], t_pkf[:])
            s_bq = s_pkf[:, 0:1]
            s_bv = s_pkf[:, 2:3]
            s_bo = s_pkf[:, 3:4]

            xj_full = per.tile([D, EP], BF16)    # resident x_j^T (later U)
            e_full = per.tile([D, EP], BF16)     # resident exp, replicated
            zparts = per.tile([D, NCH], F32)     # per-chunk Z partials

            # NOTE: a warm-up AllReduce was tried THREE times (at t~0 and
            # mid-A1): it reliably cuts the real AR's trigger->start delay
            # 11.5us -> 1.2us, but the real AR's DURATION then lands at
            # 34-43us vs a 9-38 (mean ~24) lottery without it -- all three
            # warm runs drew >=34.4 while 7/9 no-warm runs drew less.
            # Net ~+4us expected loss, so no warm-up AR.

            # ---------------- phase A1: scores ----------------
            psA_ctx = tc.tile_pool(name="psA", bufs=1, space="PSUM")
            psA = psA_ctx.__enter__()
            # PE pre-warm: dummy matmuls while the first DMAs land, so the
            # PE p-state ramps to full clock before the real stream starts.
            warm = per.tile([D, CH], BF16)
            nc.vector.memset(warm[:], 0.0)
            p_warm = psA.tile([D, CH], F32, tag="pq", bufs=3, name="p_warm")
            for i in range(12):
                nc.tensor.matmul(p_warm[:], warm[:, 128 * (i % 2):128 * (i % 2) + 128],
                                 warm[:], start=True, stop=True)

            pchain = {}      # P tiles for the deferred S matmuls

            def do_s(c):
                ps8 = psA.tile([D, CH], F32, tag="ps8", bufs=2,
                               name=f"ps8_{c}")
                nc.tensor.matmul(ps8[:], s_hrep, pchain.pop(c)[:],
                                 start=True, stop=True)
                sl1 = slice(c * CH, (c + 1) * CH)
                nc.scalar.activation(e_full[:, sl1], ps8[:], AF.Exp,
                                     bias=0.0, scale=0.25,
                                     accum_out=zparts[:, c:c + 1])

            for c in range(NCH):
                sl = slice(c * CH, (c + 1) * CH)
                if c % (XW // CH) == 0:
                    w = min(XW, EP - c * CH)
                    s_xi = wk.tile([D, XW], BF16, tag="xi", bufs=3)
                    s_ea = wk.tile([33, XW], BF16, tag="ea", bufs=3)
                    if c == 0:
                        # first batch in small leading pieces (the input DMA
                        # queue spins up at a fixed ~10us either way; this
                        # just lets chunk 0 start on the first 128KB piece)
                        pieces = [(0, CH), (CH, 2 * CH), (2 * CH, w)]
                    else:
                        pieces = [(0, w)]
                    for lo, hi in pieces:
                        psl = slice(c * CH + lo, c * CH + hi)
                        nc.sync.dma_start(s_xi[:, lo:hi], t_xiT[:, psl])
                        nc.sync.dma_start(s_ea[:, lo:hi], t_eaT[:, psl])
                        nc.sync.dma_start(xj_full[:, psl], t_xjT[:, psl])
                xsl = slice((c % (XW // CH)) * CH, (c % (XW // CH)) * CH + CH)

                p_q = psA.tile([D, CH], F32, tag="pq", bufs=3)
                nc.tensor.matmul(p_q[:], s_wq, s_xi[:, xsl], start=True, stop=True)
                p_ke = psA.tile([D, CH], F32, tag="pke", bufs=3)
                nc.tensor.matmul(p_ke[:], s_wk, xj_full[:, sl], start=True, stop=False)
                nc.tensor.matmul(p_ke[:], s_wea, s_ea[0:33, xsl], start=False, stop=True)
                # S matmul deferred by THREE chunks: gives the copy->multiply
                # chain (2 engine hops) time to finish before the PE needs it
                if c > 2:
                    do_s(c - 3)
                # KE -> SBUF copy (bias already folded into the matmul);
                # alternate ACT/DVE (GPSIMD cannot read PSUM)
                s_ke = mid.tile([D, CH], BF16, tag="ke", bufs=6)
                if c % 5 < 2:
                    nc.scalar.activation(s_ke[:], p_ke[:], AF.Identity,
                                         bias=0.0, scale=1.0)
                else:
                    nc.vector.tensor_scalar(s_ke[:], p_ke[:], 0.0, None,
                                            op0=ALU.add)
                # P = (Q + bq) * KE (DVE)
                s_p = mid.tile([D, CH], BF16, tag="p", bufs=6)
                nc.vector.scalar_tensor_tensor(s_p[:], p_q[:], s_bq, s_ke[:],
                                               op0=ALU.add, op1=ALU.mult)
                pchain[c] = s_p
            do_s(NCH - 3)
            do_s(NCH - 2)
            do_s(NCH - 1)

            # ---------------- global Z (AllReduce, hidden under A2) -------
            s_zl = per.tile([D, 1], F32)
            nc.vector.tensor_reduce(s_zl[:], zparts[:],
                                    axis=mybir.AxisListType.X, op=ALU.add)
            d_zin = dram.tile([D, 1], F32)
            d_zout = dram.tile([D, 1], F32)
            nc.sync.dma_start(d_zin[:], s_zl[:])
            # The collective blocks its issuing queue (GPSIMD) until it
            # completes, so phase A2 below must not use GPSIMD at all.
            # (An AllGather+local-sum variant was tried: same 11.5us delay,
            # duration drew at the same lottery mean -- no benefit.)
            nc.gpsimd.collective_compute(
                "AllReduce", ALU.add,
                replica_groups=[list(range(NCORES))],
                ins=[d_zin.opt()],
                outs=[d_zout.opt()],
            )

            psA_ctx.__exit__(None, None, None)
            ps2_ctx = tc.tile_pool(name="ps2", bufs=1, space="PSUM")
            ps2 = ps2_ctx.__enter__()

            # ---------------- phase A2: V and U (runs during the AR) ------
            # U = (V + bv) * e, in place over the consumed x_j chunk.
            # GPSIMD is blocked behind the collective, so split between
            # DVE (stt straight from PSUM) and ACT-copy + DVE fast bf16
            # multiply (all-16-bit DVE runs at 2x).
            for c in range(NCH):
                sl = slice(c * CH, (c + 1) * CH)
                p_v = ps2.tile([D, CH], F32, tag="pv", bufs=4)
                nc.tensor.matmul(p_v[:], s_wv, xj_full[:, sl], start=True, stop=True)
                if c % 8 < 3:
                    nc.vector.scalar_tensor_tensor(xj_full[:, sl], p_v[:],
                                                   s_bv, e_full[:, sl],
                                                   op0=ALU.add, op1=ALU.mult)
                else:
                    v_sb = mid.tile([D, CH], BF16, tag="vsb", bufs=6)
                    nc.scalar.activation(v_sb[:], p_v[:], AF.Identity,
                                         bias=s_bv, scale=1.0)
                    nc.vector.tensor_tensor(xj_full[:, sl], e_full[:, sl],
                                            v_sb[:], op=ALU.mult)

            # ---------------- finish Z -> wo2 ----------------
            s_zsum = per.tile([D, 1], F32)
            nc.sync.dma_start(s_zsum[:], d_zout[:])
            s_zc = per.tile([D, 1], F32)
            # pad cols contribute exactly exp(0)=1 each: subtract them out
            nc.vector.tensor_scalar(s_zc[:], s_zsum[:], -PADZ, None,
                                    op0=ALU.add)
            s_chd = per.tile([D, 1], F32)
            nc.vector.reciprocal(s_chd[:], s_zc[:])
            s_wo2 = per.tile([D, D], BF16)
            nc.vector.tensor_scalar(s_wo2[:], s_wo, s_chd[:], None,
                                    op0=ALU.mult)
            # identical copy at a different SBUF address: consecutive
            # same-address LDWEIGHTS appear to stall the PE ~60ns/matmul
            # (A1's weight-rotating matmuls run 375ns vs 437ns here), so
            # pass B alternates two copies of wo2
            s_wo2b = per.tile([D, D], BF16)
            nc.scalar.activation(s_wo2b[:], s_wo, AF.Copy, bias=0.0,
                                 scale=s_chd[:])

            ps2_ctx.__exit__(None, None, None)
            psB_ctx = tc.tile_pool(name="psB", bufs=1, space="PSUM")
            psB = psB_ctx.__enter__()

            # ---------------- pass B: output (pair-granularity) -----------
            # po bufs=4 (all 8 banks): 4 pairs in flight keeps the PE
            # running continuously so it ramps to full clock.
            for p in range(NPAIR):
                sl2 = slice(p * 2 * CH, (p + 1) * 2 * CH)
                p_o = psB.tile([D, 2 * CH], F32, tag="po", bufs=4,
                               name=f"po_{p}")
                for h in range(2):
                    hs = slice((2 * p + h) * CH, (2 * p + h + 1) * CH)
                    nc.tensor.matmul(p_o[:, h * CH:(h + 1) * CH],
                                     s_wo2[:] if h == 0 else s_wo2b[:],
                                     xj_full[:, hs], start=True, stop=True)
                s_o = mid.tile([D, 2 * CH], mybir.dt.float16, tag="o", bufs=6)
                if p % 2 == 0:
                    nc.scalar.activation(s_o[:], p_o[:], AF.Identity,
                                         bias=s_bo, scale=1.0)
                else:
                    nc.vector.tensor_scalar(s_o[:], p_o[:], s_bo, None,
                                            op0=ALU.add)
                nc.sync.dma_start(t_out[:, sl2], s_o[:])
            psB_ctx.__exit__(None, None, None)

    nc.compile()
    _CACHE["nc"] = nc
    return nc


def _pack_constants(wq, bq, wk, bk, wv, bv, we, be, wo, bo):
    HsumRep = np.zeros((D, D), np.float32)   # [f, hd] = (head(f)==head(hd))
    for f in range(D):
        h = f // DK
        HsumRep[f, h * DK:(h + 1) * DK] = 1.0
    pkb = np.zeros((D, 768), np.float32)
    pkb[:, 0:128] = wq
    pkb[:, 128:256] = wk
    pkb[:, 256:384] = wv
    pkb[:, 384:512] = wo
    pkb[:32, 512:640] = we
    pkb[32, 512:640] = bk + be               # ones-row weight = K/edge bias
    pkb[:, 640:768] = HsumRep
    pkf = np.zeros((D, 8), np.float32)
    pkf[:, 0] = bq
    pkf[:, 2] = bv
    pkf[:, 3] = bo
    return pkb.astype(BF), pkf


def _run(inputs, trace=False):
    x_i = np.asarray(inputs["x_i"], np.float32)
    x_j = np.asarray(inputs["x_j"], np.float32)
    ea = np.asarray(inputs["edge_attr"], np.float32)
    pkb, pkf = _pack_constants(
        np.asarray(inputs["wq"], np.float32), np.asarray(inputs["bq"], np.float32),
        np.asarray(inputs["wk"], np.float32), np.asarray(inputs["bk"], np.float32),
        np.asarray(inputs["wv"], np.float32), np.asarray(inputs["bv"], np.float32),
        np.asarray(inputs["we"], np.float32), np.asarray(inputs["be"], np.float32),
        np.asarray(inputs["wo"], np.float32), np.asarray(inputs["bo"], np.float32),
    )

    in_maps = []
    for c in range(NCORES):
        sl = slice(c * ES, (c + 1) * ES)
        xiT = np.zeros((D, EP), BF)
        xiT[:, :ES] = x_i[sl].T.astype(BF)
        xjT = np.zeros((D, EP), BF)
        xjT[:, :ES] = x_j[sl].T.astype(BF)
        eaT = np.zeros((33, EP), BF)
        eaT[:32, :ES] = ea[sl].T.astype(BF)
        eaT[32, :ES] = 1.0                   # ones-row (0 on pad cols)
        in_maps.append(dict(xiT=xiT, xjT=xjT, eaT=eaT, pkb=pkb, pkf=pkf))

    nc = _build()
    res = run_bass_kernel_spmd(nc, in_maps, list(range(NCORES)), trace=trace)

    out = np.empty((E_FULL, D), np.float32)
    for c in range(NCORES):
        sl = slice(c * ES, (c + 1) * ES)
        out[sl] = res.results[c]["outT"][:, :ES].T.astype(np.float32)
    return out, res.exec_time_ns


def kernel(**inputs) -> np.ndarray:
    return _run(inputs)[0]


# revision 40
# speedup vs baseline: 1.0798x; 1.0798x over previous
"""Trainium2 Bass kernel for nn_MultiHeadAttentionLayer (edge-wise MHA with
global softmax over the edge dimension).

Strategy (8 NeuronCores, data-parallel over edges):
  - Host shards E=250000 edges into 8 shards of 31250, zero-padded to 31744
    (62 chunks x 512), pre-transposed so features land on SBUF partitions,
    and cast to bf16.  eaT carries 33 rows: 32 edge features + a ones-row
    (1.0 on valid cols, 0.0 on pad) whose weight row is bk+be, folding the
    K/edge bias into the KE matmul for free.
  - Phase A1 (scores only, 4 matmuls/chunk): QT = wq.T@xiT, KET =
    wk.T@xjT + wea33.T@eaT (bias included).  KE copies to SBUF (ACT 2/5
    of chunks, DVE 3/5 -- DVE can read only ONE PSUM operand per op, and
    GPSIMD none), then one DVE scalar_tensor_tensor P = (QT+bq)*KET.
    S = HsumRep.T@P is deferred THREE chunks so the in-order PE never waits
    on the 2-hop copy->multiply chain (pq/pke PSUM rings are 3 deep).
    exp(S/4) per chunk -> resident SBUF bf16 e_full + per-chunk Z partials
    via the ACT accumulator.  x_j streams into a RESIDENT SBUF buffer
    (reused in A2).  Pad cols give exp(0)=1 exactly, so Z is fixed by
    subtracting the compile-time constant 8*494 after the AllReduce --
    no tail special-casing, the AR triggers ~3us after the last matmul.
  - AllReduce(add) of Z[128,1] from the GPSIMD queue (the collective
    BLOCKS its issuing queue until completion, so A2 avoids GPSIMD).
    While the AR is in flight (11.5us fixed trigger delay + 9-40us, high
    variance on this stack), phase A2 runs: VT = wv.T@xjT per chunk,
    U = (VT+bv)*e_full written IN PLACE over the resident x_j buffer.
    3/8 of chunks: DVE scalar_tensor_tensor straight from PSUM; 5/8: ACT
    copies V+bv to SBUF and DVE does the all-bf16 multiply at 2x rate.
  - 1/(Z-pad) is folded into wo's rows (wo2 = wo * chd).  Pass B (per
    chunk pair): outT = wo2.T@U + bo -> DRAM fp16; the PSUM->SBUF copy
    alternates ACT/DVE; po ring is 4 pairs deep (all 8 PSUM banks); the
    phase is output-DMA-bound (~260 GB/s per core on this stack).
  - Host gathers and transposes back to [E, 128].
  Measured: 206us (prev session baseline) -> 167-178us (AllReduce
  duration is a 9-43us lottery on this stack); rel err 2.3e-4.
  Tried and rejected: warm-up AllReduce (real AR duration balloons 2-4x),
  collective on the sync queue (BIR: Pool/DMA engines only), GPSIMD for
  any PSUM read (illegal), GPSIMD free-axis reduce (partition-axis only),
  x_j loads via GPSIMD software DGE (slower, starved A1).
"""
import os
import sys

for _p in ("/opt/trn_rl_repo", "/root/.axon_site/_ro/trn_rl_repo"):
    if os.path.isdir(_p) and _p not in sys.path:
        sys.path.append(_p)

import numpy as np
import ml_dtypes
import concourse.bacc as bacc
import concourse.tile as tile
import concourse.mybir as mybir
from concourse.bass_utils import run_bass_kernel_spmd

F32 = mybir.dt.float32
BF16 = mybir.dt.bfloat16
AF = mybir.ActivationFunctionType
ALU = mybir.AluOpType
BF = ml_dtypes.bfloat16

E_FULL = 250000
NCORES = 8
ES = E_FULL // NCORES          # 31250 edges per core
CH = 512                       # chunk size (PSUM bank width)
NCH = (ES + CH - 1) // CH      # 62 chunks
EP = NCH * CH                  # 31744 padded edges per core
D = 128
NH = 8
DK = 16
XW = 4096                      # DMA batch width (8 chunks)
NPAIR = NCH // 2               # 31 exp pairs
PADZ = float(NCORES * (EP - ES))   # exp(0)=1 per pad col: global Z excess

_CACHE = {}


def _build():
    if "nc" in _CACHE:
        return _CACHE["nc"]

    nc = bacc.Bacc(num_devices=NCORES)

    t_xiT = nc.dram_tensor("xiT", [D, EP], BF16, kind="ExternalInput")
    t_xjT = nc.dram_tensor("xjT", [D, EP], BF16, kind="ExternalInput")
    t_eaT = nc.dram_tensor("eaT", [33, EP], BF16, kind="ExternalInput")
    t_pkb = nc.dram_tensor("pkb", [D, 768], BF16, kind="ExternalInput")
    t_pkf = nc.dram_tensor("pkf", [D, 8], F32, kind="ExternalInput")
    t_out = nc.dram_tensor("outT", [D, EP], mybir.dt.float16, kind="ExternalOutput")

    with tile.TileContext(nc) as tc:
        with (
            tc.tile_pool(name="per", bufs=1) as per,      # persistent
            tc.tile_pool(name="wk", bufs=2) as wk,        # streaming loads
            tc.tile_pool(name="mid", bufs=2) as mid,      # intermediates
            tc.tile_pool(name="dram", bufs=1, space="DRAM") as dram,
        ):
            s_pkb = per.tile([D, 768], BF16)
            nc.scalar.dma_start(s_pkb[:], t_pkb[:])
            s_wq = s_pkb[:, 0:128]
            s_wk = s_pkb[:, 128:256]
            s_wv = s_pkb[:, 256:384]
            s_wo = s_pkb[:, 384:512]
            s_wea = s_pkb[0:33, 512:640]     # [we; bk+be]
            s_hrep = s_pkb[:, 640:768]       # HsumRep [f, hd]

            s_pkf = per.tile([D, 8], F32)
            nc.scalar.dma_start(s_pkf[:], t_pkf[:])
            s_bq = s_pkf[:, 0:1]
            s_bv = s_pkf[:, 2:3]
            s_bo = s_pkf[:, 3:4]

            xj_full = per.tile([D, EP], BF16)    # resident x_j^T (later U)
            e_full = per.tile([D, EP], BF16)     # resident exp, replicated
            zparts = per.tile([D, NCH], F32)     # per-chunk Z partials

            # NOTE: a warm-up AllReduce was tried THREE times (at t~0 and
            # mid-A1): it reliably cuts the real AR's trigger->start delay
            # 11.5us -> 1.2us, but the real AR's DURATION then lands at
            # 34-43us vs a 9-38 (mean ~24) lottery without it -- all three
            # warm runs drew >=34.4 while 7/9 no-warm runs drew less.
            # Net ~+4us expected loss, so no warm-up AR.

            # ---------------- phase A1: scores ----------------
            psA_ctx = tc.tile_pool(name="psA", bufs=1, space="PSUM")
            psA = psA_ctx.__enter__()
            # PE pre-warm: dummy matmuls while the first DMAs land, so the
            # PE p-state ramps to full clock before the real stream starts.
            warm = per.tile([D, CH], BF16)
            nc.vector.memset(warm[:], 0.0)
            p_warm = psA.tile([D, CH], F32, tag="pq", bufs=3, name="p_warm")
            for i in range(12):
                nc.tensor.matmul(p_warm[:], warm[:, 128 * (i % 2):128 * (i % 2) + 128],
                                 warm[:], start=True, stop=True)

            pchain = {}      # P tiles for the deferred S matmuls

            def do_s(c):
                ps8 = psA.tile([D, CH], F32, tag="ps8", bufs=2,
                               name=f"ps8_{c}")
                nc.tensor.matmul(ps8[:], s_hrep, pchain.pop(c)[:],
                                 start=True, stop=True)
                sl1 = slice(c * CH, (c + 1) * CH)
                nc.scalar.activation(e_full[:, sl1], ps8[:], AF.Exp,
                                     bias=0.0, scale=0.25,
                                     accum_out=zparts[:, c:c + 1])

            for c in range(NCH):
                sl = slice(c * CH, (c + 1) * CH)
                if c % (XW // CH) == 0:
                    w = min(XW, EP - c * CH)
                    s_xi = wk.tile([D, XW], BF16, tag="xi", bufs=3)
                    s_ea = wk.tile([33, XW], BF16, tag="ea", bufs=3)
                    if c == 0:
                        # first batch in small leading pieces (the input DMA
                        # queue spins up at a fixed ~10us either way; this
                        # just lets chunk 0 start on the first 128KB piece)
                        pieces = [(0, CH), (CH, 2 * CH), (2 * CH, w)]
                    else:
                        pieces = [(0, w)]
                    for lo, hi in pieces:
                        psl = slice(c * CH + lo, c * CH + hi)
                        nc.sync.dma_start(s_xi[:, lo:hi], t_xiT[:, psl])
                        nc.sync.dma_start(s_ea[:, lo:hi], t_eaT[:, psl])
                        nc.sync.dma_start(xj_full[:, psl], t_xjT[:, psl])
                xsl = slice((c % (XW // CH)) * CH, (c % (XW // CH)) * CH + CH)

                p_q = psA.tile([D, CH], F32, tag="pq", bufs=3)
                nc.tensor.matmul(p_q[:], s_wq, s_xi[:, xsl], start=True, stop=True)
                p_ke = psA.tile([D, CH], F32, tag="pke", bufs=3)
                nc.tensor.matmul(p_ke[:], s_wk, xj_full[:, sl], start=True, stop=False)
                nc.tensor.matmul(p_ke[:], s_wea, s_ea[0:33, xsl], start=False, stop=True)
                # S matmul deferred by THREE chunks: gives the copy->multiply
                # chain (2 engine hops) time to finish before the PE needs it
                if c > 2:
                    do_s(c - 3)
                # KE -> SBUF copy (bias already folded into the matmul);
                # alternate ACT/DVE (GPSIMD cannot read PSUM)
                s_ke = mid.tile([D, CH], BF16, tag="ke", bufs=6)
                if c % 5 < 2:
                    nc.scalar.activation(s_ke[:], p_ke[:], AF.Identity,
                                         bias=0.0, scale=1.0)
                else:
                    nc.vector.tensor_scalar(s_ke[:], p_ke[:], 0.0, None,
                                            op0=ALU.add)
                # P = (Q + bq) * KE (DVE)
                s_p = mid.tile([D, CH], BF16, tag="p", bufs=6)
                nc.vector.scalar_tensor_tensor(s_p[:], p_q[:], s_bq, s_ke[:],
                                               op0=ALU.add, op1=ALU.mult)
                pchain[c] = s_p
            do_s(NCH - 3)
            do_s(NCH - 2)
            do_s(NCH - 1)

            # ---------------- global Z (AllReduce, hidden under A2) -------
            s_zl = per.tile([D, 1], F32)
            nc.vector.tensor_reduce(s_zl[:], zparts[:],
                                    axis=mybir.AxisListType.X, op=ALU.add)
            d_zin = dram.tile([D, 1], F32)
            d_zout = dram.tile([D, 1], F32)
            nc.sync.dma_start(d_zin[:], s_zl[:])
            # The collective blocks its issuing queue (GPSIMD) until it
            # completes, so phase A2 below must not use GPSIMD at all.
            # (An AllGather+local-sum variant was tried: same 11.5us delay,
            # duration drew at the same lottery mean -- no benefit.)
            nc.gpsimd.collective_compute(
                "AllReduce", ALU.add,
                replica_groups=[list(range(NCORES))],
                ins=[d_zin.opt()],
                outs=[d_zout.opt()],
            )

            psA_ctx.__exit__(None, None, None)
            ps2_ctx = tc.tile_pool(name="ps2", bufs=1, space="PSUM")
            ps2 = ps2_ctx.__enter__()

            # ---------------- phase A2: V and U (runs during the AR) ------
            # U = (V + bv) * e, in place over the consumed x_j chunk.
            # GPSIMD is blocked behind the collective, so split between
            # DVE (stt straight from PSUM) and ACT-copy + DVE fast bf16
            # multiply (all-16-bit DVE runs at 2x).
            for c in range(NCH):
                sl = slice(c * CH, (c + 1) * CH)
                p_v = ps2.tile([D, CH], F32, tag="pv", bufs=4)
                nc.tensor.matmul(p_v[:], s_wv, xj_full[:, sl], start=True, stop=True)
                if c % 8 < 3:
                    nc.vector.scalar_tensor_tensor(xj_full[:, sl], p_v[:],
                                                   s_bv, e_full[:, sl],
                                                   op0=ALU.add, op1=ALU.mult)
                else:
                    v_sb = mid.tile([D, CH], BF16, tag="vsb", bufs=6)
                    nc.scalar.activation(v_sb[:], p_v[:], AF.Identity,
                                         bias=s_bv, scale=1.0)
                    nc.vector.tensor_tensor(xj_full[:, sl], e_full[:, sl],
                                            v_sb[:], op=ALU.mult)

            # ---------------- finish Z -> wo2 ----------------
            s_zsum = per.tile([D, 1], F32)
            nc.sync.dma_start(s_zsum[:], d_zout[:])
            s_zc = per.tile([D, 1], F32)
            # pad cols contribute exactly exp(0)=1 each: subtract them out
            nc.vector.tensor_scalar(s_zc[:], s_zsum[:], -PADZ, None,
                                    op0=ALU.add)
            s_chd = per.tile([D, 1], F32)
            nc.vector.reciprocal(s_chd[:], s_zc[:])
            s_wo2 = per.tile([D, D], BF16)
            nc.vector.tensor_scalar(s_wo2[:], s_wo, s_chd[:], None,
                                    op0=ALU.mult)
            # identical wo2 copy at a different SBUF address, alternated in
            # pass B.  Hypothesis was that same-address LDWEIGHTS stalls the
            # PE (A1's weight-rotating matmuls run 375ns vs 437ns here);
            # MEASURED REFUTED -- pass-B matmuls stay at 437ns either way.
            # Kept because it is free (ACT computes it in parallel with
            # DVE's wo2) and mathematically identical.
            s_wo2b = per.tile([D, D], BF16)
            nc.scalar.activation(s_wo2b[:], s_wo, AF.Copy, bias=0.0,
                                 scale=s_chd[:])

            ps2_ctx.__exit__(None, None, None)
            psB_ctx = tc.tile_pool(name="psB", bufs=1, space="PSUM")
            psB = psB_ctx.__enter__()

            # ---------------- pass B: output (pair-granularity) -----------
            # po bufs=4 (all 8 banks): 4 pairs in flight keeps the PE
            # running continuously so it ramps to full clock.
            for p in range(NPAIR):
                sl2 = slice(p * 2 * CH, (p + 1) * 2 * CH)
                p_o = psB.tile([D, 2 * CH], F32, tag="po", bufs=4,
                               name=f"po_{p}")
                for h in range(2):
                    hs = slice((2 * p + h) * CH, (2 * p + h + 1) * CH)
                    nc.tensor.matmul(p_o[:, h * CH:(h + 1) * CH],
                                     s_wo2[:] if h == 0 else s_wo2b[:],
                                     xj_full[:, hs], start=True, stop=True)
                s_o = mid.tile([D, 2 * CH], mybir.dt.float16, tag="o", bufs=6)
                if p % 2 == 0:
                    nc.scalar.activation(s_o[:], p_o[:], AF.Identity,
                                         bias=s_bo, scale=1.0)
                else:
                    nc.vector.tensor_scalar(s_o[:], p_o[:], s_bo, None,
                                            op0=ALU.add)
                nc.sync.dma_start(t_out[:, sl2], s_o[:])
            psB_ctx.__exit__(None, None, None)

    nc.compile()
    _CACHE["nc"] = nc
    return nc


def _pack_constants(wq, bq, wk, bk, wv, bv, we, be, wo, bo):
    HsumRep = np.zeros((D, D), np.float32)   # [f, hd] = (head(f)==head(hd))
    for f in range(D):
        h = f // DK
        HsumRep[f, h * DK:(h + 1) * DK] = 1.0
    pkb = np.zeros((D, 768), np.float32)
    pkb[:, 0:128] = wq
    pkb[:, 128:256] = wk
    pkb[:, 256:384] = wv
    pkb[:, 384:512] = wo
    pkb[:32, 512:640] = we
    pkb[32, 512:640] = bk + be               # ones-row weight = K/edge bias
    pkb[:, 640:768] = HsumRep
    pkf = np.zeros((D, 8), np.float32)
    pkf[:, 0] = bq
    pkf[:, 2] = bv
    pkf[:, 3] = bo
    return pkb.astype(BF), pkf


def _run(inputs, trace=False):
    x_i = np.asarray(inputs["x_i"], np.float32)
    x_j = np.asarray(inputs["x_j"], np.float32)
    ea = np.asarray(inputs["edge_attr"], np.float32)
    pkb, pkf = _pack_constants(
        np.asarray(inputs["wq"], np.float32), np.asarray(inputs["bq"], np.float32),
        np.asarray(inputs["wk"], np.float32), np.asarray(inputs["bk"], np.float32),
        np.asarray(inputs["wv"], np.float32), np.asarray(inputs["bv"], np.float32),
        np.asarray(inputs["we"], np.float32), np.asarray(inputs["be"], np.float32),
        np.asarray(inputs["wo"], np.float32), np.asarray(inputs["bo"], np.float32),
    )

    in_maps = []
    for c in range(NCORES):
        sl = slice(c * ES, (c + 1) * ES)
        xiT = np.zeros((D, EP), BF)
        xiT[:, :ES] = x_i[sl].T.astype(BF)
        xjT = np.zeros((D, EP), BF)
        xjT[:, :ES] = x_j[sl].T.astype(BF)
        eaT = np.zeros((33, EP), BF)
        eaT[:32, :ES] = ea[sl].T.astype(BF)
        eaT[32, :ES] = 1.0                   # ones-row (0 on pad cols)
        in_maps.append(dict(xiT=xiT, xjT=xjT, eaT=eaT, pkb=pkb, pkf=pkf))

    nc = _build()
    res = run_bass_kernel_spmd(nc, in_maps, list(range(NCORES)), trace=trace)

    out = np.empty((E_FULL, D), np.float32)
    for c in range(NCORES):
        sl = slice(c * ES, (c + 1) * ES)
        out[sl] = res.results[c]["outT"][:, :ES].T.astype(np.float32)
    return out, res.exec_time_ns


def kernel(**inputs) -> np.ndarray:
    return _run(inputs)[0]


# revision 41
# speedup vs baseline: 1.0816x; 1.0016x over previous
"""Trainium2 Bass kernel for nn_MultiHeadAttentionLayer (edge-wise MHA with
global softmax over the edge dimension).

Strategy (8 NeuronCores, data-parallel over edges):
  - Host shards E=250000 edges into 8 shards of 31250, zero-padded to 31744
    (62 chunks x 512), pre-transposed so features land on SBUF partitions,
    and cast to bf16.  eaT carries 33 rows: 32 edge features + a ones-row
    (1.0 on valid cols, 0.0 on pad) whose weight row is bk+be, folding the
    K/edge bias into the KE matmul for free.
  - Phase A1 (scores only, 4 matmuls/chunk): QT = wq.T@xiT, KET =
    wk.T@xjT + wea33.T@eaT (bias included).  KE copies to SBUF (ACT 2/5
    of chunks, DVE 3/5 -- DVE can read only ONE PSUM operand per op, and
    GPSIMD none), then one DVE scalar_tensor_tensor P = (QT+bq)*KET.
    S = HsumRep.T@P is deferred THREE chunks so the in-order PE never waits
    on the 2-hop copy->multiply chain (pq/pke PSUM rings are 3 deep).
    exp(S/4) per chunk -> resident SBUF bf16 e_full + per-chunk Z partials
    via the ACT accumulator.  x_j streams into a RESIDENT SBUF buffer
    (reused in A2).  Pad cols give exp(0)=1 exactly, so Z is fixed by
    subtracting the compile-time constant 8*494 after the AllReduce --
    no tail special-casing, the AR triggers ~3us after the last matmul.
  - AllReduce(add) of Z[128,1] from the GPSIMD queue (the collective
    BLOCKS its issuing queue until completion, so A2 avoids GPSIMD).
    While the AR is in flight (11.5us fixed trigger delay + 9-40us, high
    variance on this stack), phase A2 runs: VT = wv.T@xjT per chunk,
    U = (VT+bv)*e_full written IN PLACE over the resident x_j buffer.
    3/8 of chunks: DVE scalar_tensor_tensor straight from PSUM; 5/8: ACT
    copies V+bv to SBUF and DVE does the all-bf16 multiply at 2x rate.
  - 1/(Z-pad) is folded into wo's rows (wo2 = wo * chd).  Pass B (per
    chunk pair): outT = wo2.T@U + bo -> DRAM fp16; the PSUM->SBUF copy
    alternates ACT/DVE; po ring is 4 pairs deep (all 8 PSUM banks); the
    phase is output-DMA-bound (~260 GB/s per core on this stack).
  - Host gathers and transposes back to [E, 128].
  Measured: 206us (prev session baseline) -> 167-195us over 15 runs,
  median ~178us (the AllReduce contributes 11.5us fixed trigger delay
  plus a 9-43us duration lottery, ~26us of which A2 hides); deterministic
  core ~145us at the measured hardware floors (A1 PE-bound at 4x375ns
  per chunk, pass B at the ~260GB/s out-DMA cap).  rel err 2.3e-4.
  Tried and rejected with trace evidence: warm-up AllReduce (cuts the
  11.5us trigger delay to 1.2us but the next AR's duration then draws
  34-43us, 3/3 runs), AllGather+local-sum (same delay, same duration
  lottery), collective on the sync queue (BIR: Pool/DMA engines only),
  GPSIMD for any PSUM read (illegal), GPSIMD free-axis reduce
  (partition-axis only), x_j loads via GPSIMD software DGE (starved A1),
  alternating duplicate weights to dodge same-address LDWEIGHTS (pass-B
  matmuls stay at 437ns regardless).
"""
import os
import sys

for _p in ("/opt/trn_rl_repo", "/root/.axon_site/_ro/trn_rl_repo"):
    if os.path.isdir(_p) and _p not in sys.path:
        sys.path.append(_p)

import numpy as np
import ml_dtypes
import concourse.bacc as bacc
import concourse.tile as tile
import concourse.mybir as mybir
from concourse.bass_utils import run_bass_kernel_spmd

F32 = mybir.dt.float32
BF16 = mybir.dt.bfloat16
AF = mybir.ActivationFunctionType
ALU = mybir.AluOpType
BF = ml_dtypes.bfloat16

E_FULL = 250000
NCORES = 8
ES = E_FULL // NCORES          # 31250 edges per core
CH = 512                       # chunk size (PSUM bank width)
NCH = (ES + CH - 1) // CH      # 62 chunks
EP = NCH * CH                  # 31744 padded edges per core
D = 128
NH = 8
DK = 16
XW = 4096                      # DMA batch width (8 chunks)
NPAIR = NCH // 2               # 31 exp pairs
PADZ = float(NCORES * (EP - ES))   # exp(0)=1 per pad col: global Z excess

_CACHE = {}


def _build():
    if "nc" in _CACHE:
        return _CACHE["nc"]

    nc = bacc.Bacc(num_devices=NCORES)

    t_xiT = nc.dram_tensor("xiT", [D, EP], BF16, kind="ExternalInput")
    t_xjT = nc.dram_tensor("xjT", [D, EP], BF16, kind="ExternalInput")
    t_eaT = nc.dram_tensor("eaT", [33, EP], BF16, kind="ExternalInput")
    t_pkb = nc.dram_tensor("pkb", [D, 768], BF16, kind="ExternalInput")
    t_pkf = nc.dram_tensor("pkf", [D, 8], F32, kind="ExternalInput")
    t_out = nc.dram_tensor("outT", [D, EP], mybir.dt.float16, kind="ExternalOutput")

    with tile.TileContext(nc) as tc:
        with (
            tc.tile_pool(name="per", bufs=1) as per,      # persistent
            tc.tile_pool(name="wk", bufs=2) as wk,        # streaming loads
            tc.tile_pool(name="mid", bufs=2) as mid,      # intermediates
            tc.tile_pool(name="dram", bufs=1, space="DRAM") as dram,
        ):
            s_pkb = per.tile([D, 768], BF16)
            nc.scalar.dma_start(s_pkb[:], t_pkb[:])
            s_wq = s_pkb[:, 0:128]
            s_wk = s_pkb[:, 128:256]
            s_wv = s_pkb[:, 256:384]
            s_wo = s_pkb[:, 384:512]
            s_wea = s_pkb[0:33, 512:640]     # [we; bk+be]
            s_hrep = s_pkb[:, 640:768]       # HsumRep [f, hd]

            s_pkf = per.tile([D, 8], F32)
            nc.scalar.dma_start(s_pkf[:], t_pkf[:])
            s_bq = s_pkf[:, 0:1]
            s_bv = s_pkf[:, 2:3]
            s_bo = s_pkf[:, 3:4]

            xj_full = per.tile([D, EP], BF16)    # resident x_j^T (later U)
            e_full = per.tile([D, EP], BF16)     # resident exp, replicated
            zparts = per.tile([D, NCH], F32)     # per-chunk Z partials

            # NOTE: a warm-up AllReduce was tried THREE times (at t~0 and
            # mid-A1): it reliably cuts the real AR's trigger->start delay
            # 11.5us -> 1.2us, but the real AR's DURATION then lands at
            # 34-43us vs a 9-38 (mean ~24) lottery without it -- all three
            # warm runs drew >=34.4 while 7/9 no-warm runs drew less.
            # Net ~+4us expected loss, so no warm-up AR.

            # ---------------- phase A1: scores ----------------
            psA_ctx = tc.tile_pool(name="psA", bufs=1, space="PSUM")
            psA = psA_ctx.__enter__()
            # PE pre-warm: dummy matmuls while the first DMAs land, so the
            # PE p-state ramps to full clock before the real stream starts.
            warm = per.tile([D, CH], BF16)
            nc.vector.memset(warm[:], 0.0)
            p_warm = psA.tile([D, CH], F32, tag="pq", bufs=3, name="p_warm")
            for i in range(12):
                nc.tensor.matmul(p_warm[:], warm[:, 128 * (i % 2):128 * (i % 2) + 128],
                                 warm[:], start=True, stop=True)

            pchain = {}      # P tiles for the deferred S matmuls

            def do_s(c):
                ps8 = psA.tile([D, CH], F32, tag="ps8", bufs=2,
                               name=f"ps8_{c}")
                nc.tensor.matmul(ps8[:], s_hrep, pchain.pop(c)[:],
                                 start=True, stop=True)
                sl1 = slice(c * CH, (c + 1) * CH)
                nc.scalar.activation(e_full[:, sl1], ps8[:], AF.Exp,
                                     bias=0.0, scale=0.25,
                                     accum_out=zparts[:, c:c + 1])

            for c in range(NCH):
                sl = slice(c * CH, (c + 1) * CH)
                if c % (XW // CH) == 0:
                    w = min(XW, EP - c * CH)
                    s_xi = wk.tile([D, XW], BF16, tag="xi", bufs=3)
                    s_ea = wk.tile([33, XW], BF16, tag="ea", bufs=3)
                    if c == 0:
                        # first batch in small leading pieces (the input DMA
                        # queue spins up at a fixed ~10us either way; this
                        # just lets chunk 0 start on the first 128KB piece)
                        pieces = [(0, CH), (CH, 2 * CH), (2 * CH, w)]
                    else:
                        pieces = [(0, w)]
                    for lo, hi in pieces:
                        psl = slice(c * CH + lo, c * CH + hi)
                        nc.sync.dma_start(s_xi[:, lo:hi], t_xiT[:, psl])
                        nc.sync.dma_start(s_ea[:, lo:hi], t_eaT[:, psl])
                        nc.sync.dma_start(xj_full[:, psl], t_xjT[:, psl])
                xsl = slice((c % (XW // CH)) * CH, (c % (XW // CH)) * CH + CH)

                p_q = psA.tile([D, CH], F32, tag="pq", bufs=3)
                nc.tensor.matmul(p_q[:], s_wq, s_xi[:, xsl], start=True, stop=True)
                p_ke = psA.tile([D, CH], F32, tag="pke", bufs=3)
                nc.tensor.matmul(p_ke[:], s_wk, xj_full[:, sl], start=True, stop=False)
                nc.tensor.matmul(p_ke[:], s_wea, s_ea[0:33, xsl], start=False, stop=True)
                # S matmul deferred by THREE chunks: gives the copy->multiply
                # chain (2 engine hops) time to finish before the PE needs it
                if c > 2:
                    do_s(c - 3)
                # KE -> SBUF copy (bias already folded into the matmul);
                # alternate ACT/DVE (GPSIMD cannot read PSUM)
                s_ke = mid.tile([D, CH], BF16, tag="ke", bufs=6)
                if c % 5 < 2:
                    nc.scalar.activation(s_ke[:], p_ke[:], AF.Identity,
                                         bias=0.0, scale=1.0)
                else:
                    nc.vector.tensor_scalar(s_ke[:], p_ke[:], 0.0, None,
                                            op0=ALU.add)
                # P = (Q + bq) * KE (DVE)
                s_p = mid.tile([D, CH], BF16, tag="p", bufs=6)
                nc.vector.scalar_tensor_tensor(s_p[:], p_q[:], s_bq, s_ke[:],
                                               op0=ALU.add, op1=ALU.mult)
                pchain[c] = s_p
            do_s(NCH - 3)
            do_s(NCH - 2)
            do_s(NCH - 1)

            # ---------------- global Z (AllReduce, hidden under A2) -------
            s_zl = per.tile([D, 1], F32)
            nc.vector.tensor_reduce(s_zl[:], zparts[:],
                                    axis=mybir.AxisListType.X, op=ALU.add)
            d_zin = dram.tile([D, 1], F32)
            d_zout = dram.tile([D, 1], F32)
            nc.sync.dma_start(d_zin[:], s_zl[:])
            # The collective blocks its issuing queue (GPSIMD) until it
            # completes, so phase A2 below must not use GPSIMD at all.
            # (An AllGather+local-sum variant was tried: same 11.5us delay,
            # duration drew at the same lottery mean -- no benefit.)
            nc.gpsimd.collective_compute(
                "AllReduce", ALU.add,
                replica_groups=[list(range(NCORES))],
                ins=[d_zin.opt()],
                outs=[d_zout.opt()],
            )

            psA_ctx.__exit__(None, None, None)
            ps2_ctx = tc.tile_pool(name="ps2", bufs=1, space="PSUM")
            ps2 = ps2_ctx.__enter__()

            # ---------------- phase A2: V and U (runs during the AR) ------
            # U = (V + bv) * e, in place over the consumed x_j chunk.
            # GPSIMD is blocked behind the collective, so split between
            # DVE (stt straight from PSUM) and ACT-copy + DVE fast bf16
            # multiply (all-16-bit DVE runs at 2x).
            for c in range(NCH):
                sl = slice(c * CH, (c + 1) * CH)
                p_v = ps2.tile([D, CH], F32, tag="pv", bufs=4)
                nc.tensor.matmul(p_v[:], s_wv, xj_full[:, sl], start=True, stop=True)
                if c % 8 < 3:
                    nc.vector.scalar_tensor_tensor(xj_full[:, sl], p_v[:],
                                                   s_bv, e_full[:, sl],
                                                   op0=ALU.add, op1=ALU.mult)
                else:
                    v_sb = mid.tile([D, CH], BF16, tag="vsb", bufs=6)
                    nc.scalar.activation(v_sb[:], p_v[:], AF.Identity,
                                         bias=s_bv, scale=1.0)
                    nc.vector.tensor_tensor(xj_full[:, sl], e_full[:, sl],
                                            v_sb[:], op=ALU.mult)

            # ---------------- finish Z -> wo2 ----------------
            s_zsum = per.tile([D, 1], F32)
            nc.sync.dma_start(s_zsum[:], d_zout[:])
            s_zc = per.tile([D, 1], F32)
            # pad cols contribute exactly exp(0)=1 each: subtract them out
            nc.vector.tensor_scalar(s_zc[:], s_zsum[:], -PADZ, None,
                                    op0=ALU.add)
            s_chd = per.tile([D, 1], F32)
            nc.vector.reciprocal(s_chd[:], s_zc[:])
            s_wo2 = per.tile([D, D], BF16)
            nc.vector.tensor_scalar(s_wo2[:], s_wo, s_chd[:], None,
                                    op0=ALU.mult)
            # identical wo2 copy at a different SBUF address, alternated in
            # pass B.  Hypothesis was that same-address LDWEIGHTS stalls the
            # PE (A1's weight-rotating matmuls run 375ns vs 437ns here);
            # MEASURED REFUTED -- pass-B matmuls stay at 437ns either way.
            # Kept because it is free (ACT computes it in parallel with
            # DVE's wo2) and mathematically identical.
            s_wo2b = per.tile([D, D], BF16)
            nc.scalar.activation(s_wo2b[:], s_wo, AF.Copy, bias=0.0,
                                 scale=s_chd[:])

            ps2_ctx.__exit__(None, None, None)
            psB_ctx = tc.tile_pool(name="psB", bufs=1, space="PSUM")
            psB = psB_ctx.__enter__()

            # ---------------- pass B: output (pair-granularity) -----------
            # po bufs=4 (all 8 banks): 4 pairs in flight keeps the PE
            # running continuously so it ramps to full clock.
            for p in range(NPAIR):
                sl2 = slice(p * 2 * CH, (p + 1) * 2 * CH)
                p_o = psB.tile([D, 2 * CH], F32, tag="po", bufs=4,
                               name=f"po_{p}")
                for h in range(2):
                    hs = slice((2 * p + h) * CH, (2 * p + h + 1) * CH)
                    nc.tensor.matmul(p_o[:, h * CH:(h + 1) * CH],
                                     s_wo2[:] if h == 0 else s_wo2b[:],
                                     xj_full[:, hs], start=True, stop=True)
                s_o = mid.tile([D, 2 * CH], mybir.dt.float16, tag="o", bufs=6)
                if p % 2 == 0:
                    nc.scalar.activation(s_o[:], p_o[:], AF.Identity,
                                         bias=s_bo, scale=1.0)
                else:
                    nc.vector.tensor_scalar(s_o[:], p_o[:], s_bo, None,
                                            op0=ALU.add)
                nc.sync.dma_start(t_out[:, sl2], s_o[:])
            psB_ctx.__exit__(None, None, None)

    nc.compile()
    _CACHE["nc"] = nc
    return nc


def _pack_constants(wq, bq, wk, bk, wv, bv, we, be, wo, bo):
    HsumRep = np.zeros((D, D), np.float32)   # [f, hd] = (head(f)==head(hd))
    for f in range(D):
        h = f // DK
        HsumRep[f, h * DK:(h + 1) * DK] = 1.0
    pkb = np.zeros((D, 768), np.float32)
    pkb[:, 0:128] = wq
    pkb[:, 128:256] = wk
    pkb[:, 256:384] = wv
    pkb[:, 384:512] = wo
    pkb[:32, 512:640] = we
    pkb[32, 512:640] = bk + be               # ones-row weight = K/edge bias
    pkb[:, 640:768] = HsumRep
    pkf = np.zeros((D, 8), np.float32)
    pkf[:, 0] = bq
    pkf[:, 2] = bv
    pkf[:, 3] = bo
    return pkb.astype(BF), pkf


def _run(inputs, trace=False):
    x_i = np.asarray(inputs["x_i"], np.float32)
    x_j = np.asarray(inputs["x_j"], np.float32)
    ea = np.asarray(inputs["edge_attr"], np.float32)
    pkb, pkf = _pack_constants(
        np.asarray(inputs["wq"], np.float32), np.asarray(inputs["bq"], np.float32),
        np.asarray(inputs["wk"], np.float32), np.asarray(inputs["bk"], np.float32),
        np.asarray(inputs["wv"], np.float32), np.asarray(inputs["bv"], np.float32),
        np.asarray(inputs["we"], np.float32), np.asarray(inputs["be"], np.float32),
        np.asarray(inputs["wo"], np.float32), np.asarray(inputs["bo"], np.float32),
    )

    in_maps = []
    for c in range(NCORES):
        sl = slice(c * ES, (c + 1) * ES)
        xiT = np.zeros((D, EP), BF)
        xiT[:, :ES] = x_i[sl].T.astype(BF)
        xjT = np.zeros((D, EP), BF)
        xjT[:, :ES] = x_j[sl].T.astype(BF)
        eaT = np.zeros((33, EP), BF)
        eaT[:32, :ES] = ea[sl].T.astype(BF)
        eaT[32, :ES] = 1.0                   # ones-row (0 on pad cols)
        in_maps.append(dict(xiT=xiT, xjT=xjT, eaT=eaT, pkb=pkb, pkf=pkf))

    nc = _build()
    res = run_bass_kernel_spmd(nc, in_maps, list(range(NCORES)), trace=trace)

    out = np.empty((E_FULL, D), np.float32)
    for c in range(NCORES):
        sl = slice(c * ES, (c + 1) * ES)
        out[sl] = res.results[c]["outT"][:, :ES].T.astype(np.float32)
    return out, res.exec_time_ns


def kernel(**inputs) -> np.ndarray:
    return _run(inputs)[0]
